# revision 1
# baseline (speedup 1.0000x reference)
"""Trainium2 Bass kernel for nn_AttentionBlock (sparse_attention).

Full-input contract: kernel(**inputs) takes the complete tensors and returns
the complete [4, 512, 512] output. Internally shards over 8 NeuronCores as
(batch, i-half): core c handles batch c//2, query rows (c%2)*256 ..+256.

bf16 matmul pipeline. Host pre-processing: refCov is pre-permuted into the
on-chip tile layout and cast to bf16 (halves HBM traffic and removes the
on-chip repack entirely); weights / x / small constants are host-cast to
bf16 where used as matmul operands. Softmax runs without max-subtraction
(logits are O(1)); j-masking is folded into the score matmul (K=1 bias-row
accumulate) and the repr-MLP L2 evacuation; invalid-i rows are fixed with
one per-partition tensor_scalar; layernorm rsqrt runs on DVE (magic +
Newton) so ACT keeps a single function table. refc streams in 16-row
chunks with 2-chunk DMA prefetch; deinterleave DMAs issue from the (idle)
GPSIMD engine so they never block the SP DMA queue.

Self-contained: hardcodes all shapes; no sibling imports.
"""

import sys

if "/opt/trn_rl_repo" not in sys.path:
    sys.path.insert(0, "/opt/trn_rl_repo")

from collections import deque
from contextlib import ExitStack

import ml_dtypes
import numpy as np

import concourse.bass as bass
import concourse.tile as tile
from concourse import bacc, mybir
from concourse.bass_utils import run_bass_kernel_spmd
from concourse.masks import make_identity

F32 = mybir.dt.float32
BF16 = mybir.dt.bfloat16
I32 = mybir.dt.int32
AF = mybir.ActivationFunctionType
ALU = mybir.AluOpType
AX = mybir.AxisListType

BF = ml_dtypes.bfloat16
NEG = -1.0e30

B, L_FULL, H, NH = 4, 512, 512, 8
DK = H // NH  # 64
CIN, CHID = 53, 32
N_CORES = 8
PRE = 3  # refc chunk DMA prefetch depth


def build_program(L, LI, has_bq, has_bk, has_bv, has_bo, has_r2b, r2b_vals,
                  trace_sim=False):
    """One-core program: attention block over LI query rows, L context."""
    assert L % 128 == 0 and LI % 16 == 0
    NJB = L // 128            # j blocks
    IBS = min(128, LI)        # i-block size for attention tiles
    NIB = LI // IBS           # i-blocks
    NHC = H // 128            # h chunks (4)
    NCH = LI // 16            # refc chunks (16 i-rows each)
    CPI = IBS // 16           # chunks per i-block
    CB_F = 8 * NJB * 2 * CIN  # free elems per c2b partition row
    BIGF = max(L, H)          # "big" psum tile free size
    scale = float(1.0 / np.sqrt(DK))

    nc = bacc.Bacc()

    xb_d = nc.dram_tensor("xb", [L, H], BF16, kind="ExternalInput")
    xqb_d = nc.dram_tensor("xqb", [LI, H], BF16, kind="ExternalInput")
    xq_d = nc.dram_tensor("xq", [LI, H], F32, kind="ExternalInput")
    refcb_d = nc.dram_tensor("refcb", [NCH * 128, CB_F], BF16, kind="ExternalInput")
    wqkvo_d = nc.dram_tensor("wqkvo", [H, 4 * H], BF16, kind="ExternalInput")
    bqkvo_d = nc.dram_tensor("bqkvo", [H, 4], F32, kind="ExternalInput")
    bd1_d = nc.dram_tensor("bd1h", [2 * CIN, 2 * CHID], BF16, kind="ExternalInput")
    bd2_d = nc.dram_tensor("bd2h", [128, 4 * NH], BF16, kind="ExternalInput")
    r1b4_d = nc.dram_tensor("r1b4h", [128, 1], F32, kind="ExternalInput")
    jbias_d = nc.dram_tensor("jbias", [1, L], BF16, kind="ExternalInput")
    mbias_d = nc.dram_tensor("mbias", [L, 1], F32, kind="ExternalInput")
    miof_d = nc.dram_tensor("miof", [LI, 2], F32, kind="ExternalInput")
    lng_d = nc.dram_tensor("lng", [H, 1], F32, kind="ExternalInput")
    lnb_d = nc.dram_tensor("lnb", [H, 1], F32, kind="ExternalInput")
    y_d = nc.dram_tensor("y", [LI, H], F32, kind="ExternalOutput")

    with tile.TileContext(nc, trace_sim=trace_sim) as tc, ExitStack() as ctx:
        P = ctx.enter_context(tc.tile_pool(name="persist", bufs=1))
        pc2b = ctx.enter_context(tc.tile_pool(name="c2b", bufs=PRE + 2))
        ptrs = ctx.enter_context(tc.tile_pool(name="trs", bufs=5))
        phid = ctx.enter_context(tc.tile_pool(name="hid", bufs=3))
        pl2s = ctx.enter_context(tc.tile_pool(name="l2s", bufs=3))
        pea = ctx.enter_context(tc.tile_pool(name="ea", bufs=4))
        per = ctx.enter_context(tc.tile_pool(name="er", bufs=5))
        pwts = ctx.enter_context(tc.tile_pool(name="wts", bufs=4))
        pry = ctx.enter_context(tc.tile_pool(name="ry", bufs=2))
        psc = ctx.enter_context(tc.tile_pool(name="sc", bufs=6))
        # PSUM pools (2KB banks): big 2 + tr 2 + l1 2 + l2 1 + wav 1 = 8
        pp_big = ctx.enter_context(tc.tile_pool(name="ppbig", bufs=2, space="PSUM"))
        pp_tr = ctx.enter_context(tc.tile_pool(name="pptr", bufs=2, space="PSUM"))
        pp_l1 = ctx.enter_context(tc.tile_pool(name="ppl1", bufs=2, space="PSUM"))
        pp_l2 = ctx.enter_context(tc.tile_pool(name="ppl2", bufs=1, space="PSUM"))
        pp_wav = ctx.enter_context(tc.tile_pool(name="ppwav", bufs=1, space="PSUM"))

        # ---------- phase 0: identity + x loads first on the DMA queue ------
        ident = P.tile([128, 128], F32, tag="ident")
        make_identity(nc, ident)
        identb = P.tile([128, 128], BF16, tag="identb")
        nc.vector.tensor_copy(out=identb, in_=ident)

        xf = []
        for t in range(NJB):
            # row p of tile t = x row j = NJB*p + t (pi-permuted j order)
            xt = P.tile([128, H], BF16, tag=f"xf{t}")
            nc.sync.dma_start(out=xt, in_=xb_d[t : L : NJB, :])
            xf.append(xt)
        xq_sb = []
        xqb_sb = []
        for ib in range(NIB):
            xt = P.tile([IBS, H], BF16, tag=f"xqb{ib}")
            nc.sync.dma_start(out=xt, in_=xqb_d[IBS * ib : IBS * (ib + 1), :])
            xqb_sb.append(xt)
            xt = P.tile([IBS, H], F32, tag=f"xq{ib}")
            nc.sync.dma_start(out=xt, in_=xq_d[IBS * ib : IBS * (ib + 1), :])
            xq_sb.append(xt)

        # ---------- refc chunk prefetch machinery ----------
        c2b_q = deque()

        def issue_c2b(c):
            t = pc2b.tile([128, 8, NJB, 2, CIN], BF16, tag="c2b", name="c2b")
            nc.sync.dma_start(
                out=t.rearrange("p a k i c -> p (a k i c)"),
                in_=refcb_d[128 * c : 128 * (c + 1), :],
            )
            c2b_q.append(t)

        for c in range(min(PRE, NCH)):
            issue_c2b(c)

        # ---------- weights + small constants (all pre-cast on host) --------
        def w_sb(nm, kk):
            wi = ("wq", "wk", "wv", "wo").index(nm)
            return wbig[kk][:, H * wi : H * (wi + 1)]

        bd1b = P.tile([2 * CIN, 2 * CHID], BF16, tag="bd1b")
        nc.sync.dma_start(out=bd1b, in_=bd1_d[:, :])
        bd2b = P.tile([128, 4 * NH], BF16, tag="bd2b")
        nc.sync.dma_start(out=bd2b, in_=bd2_d[:, :])
        r1b4 = P.tile([128, 1], F32, tag="r1b4")
        nc.sync.dma_start(out=r1b4, in_=r1b4_d[:, :])

        jbias_b = P.tile([1, L], BF16, tag="jbias_b")
        nc.sync.dma_start(out=jbias_b, in_=jbias_d[:, :])
        ones1b = P.tile([1, 128], BF16, tag="ones1b")
        nc.gpsimd.memset(ones1b, 1.0)

        mbias128 = P.tile([128, L], F32, tag="mbias128")
        nc.sync.dma_start(
            out=mbias128, in_=bass.AP(tensor=mbias_d, offset=0, ap=[[0, 128], [1, L]])
        )
        miof = []
        for ib in range(NIB):
            t = P.tile([IBS, 2], F32, tag=f"miof{ib}")
            nc.sync.dma_start(out=t, in_=miof_d[IBS * ib : IBS * (ib + 1), :])
            miof.append(t)

        g_bc = P.tile([128, H], F32, tag="g_bc")
        nc.sync.dma_start(
            out=g_bc, in_=bass.AP(tensor=lng_d, offset=0, ap=[[0, 128], [1, H]])
        )
        b_bc = P.tile([128, H], F32, tag="b_bc")
        nc.sync.dma_start(
            out=b_bc, in_=bass.AP(tensor=lnb_d, offset=0, ap=[[0, 128], [1, H]])
        )
        bo_bc = None
        if has_bo:
            bo_d2 = nc.dram_tensor("bo2", [H, 1], F32, kind="ExternalInput")
            bo_bc = P.tile([128, H], F32, tag="bo_bc")
            nc.sync.dma_start(
                out=bo_bc, in_=bass.AP(tensor=bo_d2, offset=0, ap=[[0, 128], [1, H]])
            )
        r2b128 = None
        if has_r2b:
            r2b128 = P.tile([128, 1], F32, tag="r2b128")
            r2bh_d = nc.dram_tensor("r2b128h", [128, 1], F32, kind="ExternalInput")
            nc.sync.dma_start(out=r2b128, in_=r2bh_d[:, :])

        bias_sb = {}
        if has_bq or has_bk or has_bv:
            bqkvo = P.tile([128, NHC, 4], F32, tag="bqkvo")
            nc.sync.dma_start(
                out=bqkvo, in_=bqkvo_d[:, :].rearrange("(c p) b -> p c b", p=128)
            )
            for wi, nm in enumerate(("bq", "bk", "bv")):
                for kk in range(NHC):
                    bias_sb[(nm, kk)] = bqkvo[:, kk, wi : wi + 1]

        wbig = []
        for kk in range(NHC):
            t = P.tile([128, 4 * H], BF16, tag=f"wbig{kk}")
            nc.sync.dma_start(out=t, in_=wqkvo_d[128 * kk : 128 * (kk + 1), :])
            wbig.append(t)

        # ---------- x transposes ----------
        xT = []
        for hc in range(NHC):
            ps = pp_tr.tile([128, L], BF16, tag="trp", name="psT")
            for jt in range(NJB):
                nc.tensor.transpose(
                    out=ps[:, 128 * jt : 128 * (jt + 1)],
                    in_=xf[jt][:, 128 * hc : 128 * (hc + 1)],
                    identity=identb,
                )
            xs = P.tile([128, L], BF16, tag=f"xT{hc}")
            nc.vector.tensor_copy(out=xs, in_=ps)
            xT.append(xs)
        xqT = []
        for hc in range(NHC):
            ps = pp_tr.tile([128, L], BF16, tag="trp", name="psT")
            for ib in range(NIB):
                nc.tensor.transpose(
                    out=ps[:, IBS * ib : IBS * (ib + 1)],
                    in_=xqb_sb[ib][:, 128 * hc : 128 * (hc + 1)],
                    identity=identb[0:IBS, 0:IBS],
                )
            xs = P.tile([128, LI], BF16, tag=f"xqT{hc}")
            nc.vector.tensor_copy(out=xs, in_=ps[:, 0:LI])
            xqT.append(xs)

        # ---------- q/k/v projections (bf16), emitted after chunk 0 ------
        qT, kT, v_sb = [], [], []

        def emit_qkv():
            for t in range(NHC):
                ps = pp_big.tile([128, BIGF], F32, tag="big", name="psq")
                for kk in range(NHC):
                    nc.tensor.matmul(
                        out=ps[:, 0:LI],
                        lhsT=w_sb("wq", kk)[:, 128 * t : 128 * (t + 1)],
                        rhs=xqT[kk], start=(kk == 0), stop=(kk == NHC - 1),
                    )
                s = P.tile([128, LI], BF16, tag=f"qT{t}")
                if has_bq:
                    nc.scalar.activation(out=s, in_=ps[:, 0:LI], func=AF.Identity,
                                         bias=bias_sb[("bq", t)])
                else:
                    nc.scalar.copy(out=s, in_=ps[:, 0:LI])
                qT.append(s)
            for t in range(NHC):
                ps = pp_big.tile([128, BIGF], F32, tag="big", name="psk")
                for kk in range(NHC):
                    nc.tensor.matmul(
                        out=ps[:, 0:L],
                        lhsT=w_sb("wk", kk)[:, 128 * t : 128 * (t + 1)],
                        rhs=xT[kk], start=(kk == 0), stop=(kk == NHC - 1),
                    )
                s = P.tile([128, L], BF16, tag=f"kT{t}")
                if has_bk:
                    nc.scalar.activation(out=s, in_=ps[:, 0:L], func=AF.Identity,
                                         bias=bias_sb[("bk", t)])
                else:
                    nc.vector.tensor_copy(out=s, in_=ps[:, 0:L])
                kT.append(s)
            for t in range(NJB):
                ps = pp_big.tile([128, BIGF], F32, tag="big", name="psv")
                for kk in range(NHC):
                    nc.tensor.matmul(
                        out=ps[:, 0:H],
                        lhsT=xT[kk][:, 128 * t : 128 * (t + 1)],
                        rhs=w_sb("wv", kk),
                        start=(kk == 0), stop=(kk == NHC - 1),
                    )
                s = P.tile([128, H], BF16, tag=f"v{t}")
                nc.scalar.copy(out=s, in_=ps[:, 0:H])  # bv folded into avT evac
                v_sb.append(s)

        refS = [
            P.tile([IBS, NH, L], F32, tag=f"refS{ib}", name=f"refS{ib}")
            for ib in range(NIB)
        ]
        aoT = [P.tile([128, LI], BF16, tag=f"aoT{t}", name=f"aoT{t}")
               for t in range(NHC)]

        st = {"l1p": None, "l2p": None}
        pending_deint = []

        def flush_deints():
            # split between GPSIMD (SWDGE) and SP (HWDGE) so neither device
            # becomes the bottleneck; deferral keeps SP head-of-line safe
            while pending_deint:
                ib, u, l2s = pending_deint.pop(0)
                for nh in range(NH):
                    eng = nc.gpsimd if (u + nh) % 2 == 0 else nc.sync
                    eng.dma_start(
                        out=refS[ib][16 * u : 16 * u + 16, nh, :],
                        in_=l2s[nh : 128 : 8, :],
                    )

        def emit_chunk(c):
            """16 i-rows: prefetch DMA + deferred deint + transposes + MLP."""
            ib = (16 * c) // IBS
            if c + PRE < NCH:
                issue_c2b(c + PRE)
            flush_deints()
            c2b = c2b_q.popleft()
            for pr2 in range(4):
                m = 4 * c + pr2         # 2-pair unit (4 i-rows)
                # 8 transposes (2 pairs) into one full psum bank
                trp = pp_tr.tile([2 * CIN, 2 * L], BF16, tag="trp")
                for half in range(2):
                    pr = 2 * pr2 + half
                    for jb in range(NJB):
                        nc.tensor.transpose(
                            out=trp[:, L * half + 128 * jb : L * half + 128 * (jb + 1)],
                            in_=c2b[:, pr, jb, :, :].rearrange("p i c -> p (i c)"),
                            identity=identb,
                        )
                trs = ptrs.tile([2 * CIN, 2 * L], BF16, tag="trs")
                nc.vector.tensor_copy(out=trs, in_=trp)
                l1p = pp_l1.tile([128, BIGF], F32, tag="l1", name="l1p")[:, 0:L]
                for half in range(2):
                    nc.tensor.matmul(
                        out=l1p[64 * half : 64 * half + 64, :],
                        lhsT=bd1b, rhs=trs[:, L * half : L * (half + 1)],
                        start=True, stop=True,
                    )
                hid = phid.tile([128, L], BF16, tag="hid")
                nc.scalar.activation(out=hid, in_=l1p, func=AF.Relu, bias=r1b4)
                q4 = m % 4
                if q4 == 0:
                    st["l2p"] = pp_l2.tile([128, L], F32, tag="l2", name="l2p")
                nc.tensor.matmul(
                    out=st["l2p"][32 * q4 : 32 * q4 + 32, :],
                    lhsT=bd2b, rhs=hid, start=True, stop=True,
                    tile_position=(0, 32 * q4),
                )
                if q4 == 3:
                    u = (m // 4) % (IBS // 16)
                    l2s = pl2s.tile([128, L], F32, tag="l2s")
                    if has_r2b:
                        nc.vector.tensor_scalar_add(
                            out=l2s, in0=st["l2p"], scalar1=r2b128,
                        )
                        nc.vector.tensor_add(out=l2s, in0=l2s, in1=mbias128)
                    else:
                        nc.vector.tensor_add(out=l2s, in0=st["l2p"], in1=mbias128)
                    pending_deint.append((ib, u, l2s))

        eaP = [P.tile([IBS, L], BF16, tag=f"eaP{nh}", name=f"eaP{nh}")
               for nh in range(NH)]

        def emit_head_attn(ib, nh, store=False):
            """Scores + exp + 1/sa scaling; refS-independent."""
            t, s = nh // 2, nh % 2
            sp = pp_big.tile([IBS, BIGF], F32, tag="big", name="sp")[:, 0:L]
            nc.tensor.matmul(
                out=sp,
                lhsT=qT[t][64 * s : 64 * s + 64, IBS * ib : IBS * (ib + 1)],
                rhs=kT[t][64 * s : 64 * s + 64, :],
                start=True, stop=False,
            )
            nc.tensor.matmul(
                out=sp, lhsT=ones1b[:, 0:IBS], rhs=jbias_b,
                start=False, stop=True, skip_group_check=True,
            )
            ea_t = eaP[nh] if store else pea.tile([IBS, L], BF16, tag="ea")
            sa = psc.tile([IBS, 1], F32, tag="sa")
            nc.scalar.activation(
                out=ea_t, in_=sp, func=AF.Exp, bias=0.0, scale=scale, accum_out=sa
            )
            isa = psc.tile([IBS, 1], F32, tag="isa")
            nc.vector.reciprocal(out=isa, in_=sa)
            # fold the invalid-i handling here: w = ea*(isa*mi) + (1-mi)*2/L ...
            nc.vector.tensor_mul(out=isa, in0=isa, in1=miof[ib][:, 0:1])
            nc.vector.tensor_scalar(
                out=ea_t, in0=ea_t, scalar1=isa, scalar2=miof[ib][:, 1:2],
                op0=ALU.mult, op1=ALU.add,
            )
            return ea_t

        def emit_head_ref_a(ib, nh, ea_t):
            """er exp + combine (ACT/DVE only); needs refS[ib] complete."""
            er_t = per.tile([IBS, L], BF16, tag="er")
            sr = psc.tile([IBS, 1], F32, tag="sr")
            nc.scalar.activation(
                out=er_t, in_=refS[ib][:, nh, :], func=AF.Exp, bias=0.0, scale=1.0,
                accum_out=sr,
            )
            isr = psc.tile([IBS, 1], F32, tag="isr")
            nc.vector.reciprocal(out=isr, in_=sr)
            # ... + er*(isr*mi); 0.5 folded into avT evac
            nc.vector.tensor_mul(out=isr, in0=isr, in1=miof[ib][:, 0:1])
            nc.vector.tensor_scalar_mul(out=er_t, in0=er_t, scalar1=isr)
            nc.vector.tensor_add(out=er_t, in0=er_t, in1=ea_t)
            return er_t

        def emit_head_ref_b(ib, nh, ea_t, alt=False):
            """wT transpose + AV (PE heavy)."""
            t, s = nh // 2, nh % 2
            odd = alt and (nh % 2 == 1)
            wtp = (pp_tr if odd else pp_wav).tile(
                [128, NJB * IBS], BF16, tag=("trp" if odd else "wav"), name="wtp")
            for k in range(NJB):
                nc.tensor.transpose(
                    out=wtp[:, IBS * k : IBS * (k + 1)],
                    in_=ea_t[:, 128 * k : 128 * (k + 1)],
                    identity=identb[0:IBS, 0:IBS],
                )
            wts = pwts.tile([128, NJB * IBS], BF16, tag="wts")
            nc.vector.tensor_copy(out=wts, in_=wtp)
            avp = (pp_tr if odd else pp_wav).tile(
                [64, IBS], F32, tag=("trp" if odd else "wav"), name="avp")
            for k in range(NJB):
                nc.tensor.matmul(
                    out=avp,
                    lhsT=v_sb[k][:, 64 * nh : 64 * nh + 64],
                    rhs=wts[:, IBS * k : IBS * (k + 1)],
                    start=(k == 0), stop=(k == NJB - 1),
                )
            if has_bv:
                nc.scalar.activation(
                    out=aoT[t][64 * s : 64 * s + 64, IBS * ib : IBS * (ib + 1)],
                    in_=avp, func=AF.Identity, scale=0.5,
                    bias=bias_sb[("bv", t)][64 * s : 64 * s + 64, :],
                )
            else:
                nc.scalar.activation(
                    out=aoT[t][64 * s : 64 * s + 64, IBS * ib : IBS * (ib + 1)],
                    in_=avp, func=AF.Copy, bias=0.0, scale=0.5,
                )

        def rsqrt_dve(out, v):
            """out = 1/sqrt(v) on DVE only (magic seed + Newton steps)."""
            yb = psc.tile([IBS, 1], I32, tag="rsq_i")
            nc.vector.tensor_scalar(
                out=yb, in0=v.bitcast(I32), scalar1=1, scalar2=None,
                op0=ALU.logical_shift_right,
            )
            nc.vector.tensor_scalar(
                out=yb, in0=yb, scalar1=-1, scalar2=0x5F3759DF,
                op0=ALU.mult, op1=ALU.add,
            )
            y = yb.bitcast(F32)
            t2 = psc.tile([IBS, 1], F32, tag="rsq_t")
            for _ in range(2):
                nc.vector.tensor_mul(out=t2, in0=y, in1=y)
                nc.vector.tensor_mul(out=t2, in0=t2, in1=v)
                nc.vector.tensor_scalar(
                    out=t2, in0=t2, scalar1=-0.5, scalar2=1.5,
                    op0=ALU.mult, op1=ALU.add,
                )
                nc.vector.tensor_mul(out=y, in0=y, in1=t2)
            nc.vector.tensor_copy(out=out, in_=y)

        def emit_proj(ib):
            pp = pp_big.tile([IBS, BIGF], F32, tag="big", name="pp")
            for kk in range(NHC):
                nc.tensor.matmul(
                    out=pp[:, 0:H],
                    lhsT=aoT[kk][:, IBS * ib : IBS * (ib + 1)],
                    rhs=w_sb("wo", kk),
                    start=(kk == 0), stop=(kk == NHC - 1),
                )
            r_t = pry.tile([IBS, H], F32, tag="rt")
            nc.vector.scalar_tensor_tensor(
                out=r_t, in0=pp[:, 0:H], scalar=1.0, in1=xq_sb[ib],
                op0=ALU.mult, op1=ALU.add,
            )
            if has_bo:
                nc.vector.tensor_add(out=r_t, in0=r_t, in1=bo_bc[0:IBS, :])
            stats = psc.tile([IBS, 6], F32, tag="stats")
            nc.vector.bn_stats(out=stats, in_=r_t)
            mv = psc.tile([IBS, 2], F32, tag="mv")
            nc.vector.bn_aggr(out=mv, in_=stats)
            veps = psc.tile([IBS, 1], F32, tag="veps")
            nc.vector.tensor_scalar_add(out=veps, in0=mv[:, 1:2], scalar1=1e-5)
            rstd = psc.tile([IBS, 1], F32, tag="rstd")
            rsqrt_dve(rstd, veps)
            nc.vector.tensor_scalar(
                out=r_t, in0=r_t, scalar1=mv[:, 0:1], scalar2=rstd,
                op0=ALU.subtract, op1=ALU.mult,
            )
            y_t = pry.tile([IBS, H], F32, tag="yt")
            nc.vector.tensor_mul(out=y_t, in0=r_t, in1=g_bc[0:IBS, :])
            nc.vector.tensor_add(out=y_t, in0=y_t, in1=b_bc[0:IBS, :])
            nc.scalar.dma_start(out=y_d[IBS * ib : IBS * (ib + 1), :], in_=y_t)

        # ---------- schedule ----------
        emit_chunk(0)
        emit_qkv()
        for c in range(1, CPI):
            emit_chunk(c)
        last = NIB - 1
        for ib in range(NIB):
            nxt = list(range(CPI * (ib + 1), min(CPI * (ib + 2), NCH)))
            flush_deints()  # refS[ib] writes must precede head reads
            alt = not nxt
            for nh in range(NH):
                if ib == last and NIB > 1:
                    ea_t = eaP[nh]
                else:
                    ea_t = emit_head_attn(ib, nh)
                w_t = emit_head_ref_a(ib, nh, ea_t)
                if nxt:
                    emit_chunk(nxt.pop(0))
                emit_head_ref_b(ib, nh, w_t, alt=alt)
                if ib + 1 == last and NIB > 1:
                    # precompute the final i-block's attn side early
                    emit_head_attn(last, nh, store=True)
            while nxt:
                emit_chunk(nxt.pop(0))
            if ib == last and NIB > 1:
                # ref side only; ea tiles were precomputed
                pass
            emit_proj(ib)
        flush_deints()

    nc.compile()
    return nc


def _make_bd1(r1w):
    bd1 = np.zeros((2 * CIN, 2 * CHID), np.float32)
    bd1[0:CIN, 0:CHID] = r1w
    bd1[CIN : 2 * CIN, CHID : 2 * CHID] = r1w
    return bd1


def _make_bd2(r2w):
    bd2 = np.zeros((128, 4 * NH), np.float32)
    for g in range(4):
        bd2[32 * g : 32 * g + CHID, NH * g : NH * (g + 1)] = r2w
    return bd2


def _make_r1b4(r1b):
    r1b4 = np.zeros((128, 1), np.float32)
    for g in range(4):
        r1b4[32 * g : 32 * g + CHID, 0] = r1b
    return r1b4


def _pack_refc(rc, L):
    """[LI, L, CIN] f32 -> [(LI/16)*128, 8*NJB*2*CIN] bf16 in c2b tile layout."""
    LI = rc.shape[0]
    NJB = L // 128
    nch = LI // 16
    a = rc.reshape(nch, 8, 2, 128, NJB, CIN)   # (c, i2, i, p, k, cc)
    a = a.transpose(0, 3, 1, 4, 2, 5)          # (c, p, i2, k, i, cc)
    return np.ascontiguousarray(
        a.reshape(nch * 128, 8 * NJB * 2 * CIN).astype(BF)
    )


_PROG_CACHE = {}


def _get_program(L, LI, flags, r2b_vals):
    key = (L, LI, flags)
    if key not in _PROG_CACHE:
        _PROG_CACHE[key] = build_program(L, LI, *flags, r2b_vals)
    return _PROG_CACHE[key]


def make_in_maps(x, mask, refCov, wq, bq, wk, bk, wv, bv, wo, bo,
                 r1w, r1b, r2w, r2b, ln_g, ln_b, n_cores=N_CORES, LI=None):
    Bc, L, Hc = x.shape
    if LI is None:
        LI = (Bc * L) // n_cores
    f = np.float32
    shared = {
        "wqkvo": np.ascontiguousarray(
            np.concatenate([np.asarray(w, f) for w in (wq, wk, wv, wo)], axis=1)
        ).astype(BF),
        "bqkvo": np.ascontiguousarray(
            np.stack([np.asarray(b, f) for b in (bq, bk, bv, bo)], axis=1)
        ),
        "bd1h": _make_bd1(np.asarray(r1w, f)).astype(BF),
        "bd2h": _make_bd2(np.asarray(r2w, f)).astype(BF),
        "r1b4h": _make_r1b4(np.asarray(r1b, f)),
        "lng": np.ascontiguousarray(ln_g, f).reshape(Hc, 1),
        "lnb": np.ascontiguousarray(ln_b, f).reshape(Hc, 1),
    }
    njb = L // 128
    pidx = (np.arange(L) % 128) * njb + np.arange(L) // 128
    per_batch = L // LI
    in_maps = []
    for c in range(n_cores):
        b, half = c // per_batch, c % per_batch
        i0 = half * LI
        m = dict(shared)
        xb = np.asarray(x[b], f)
        m["xb"] = np.ascontiguousarray(xb).astype(BF)
        m["xqb"] = np.ascontiguousarray(xb[i0 : i0 + LI]).astype(BF)
        m["xq"] = np.ascontiguousarray(xb[i0 : i0 + LI])
        m["refcb"] = _pack_refc(np.asarray(refCov[b, i0 : i0 + LI], f), L)
        mp = np.asarray(mask[b][pidx], f)          # permuted j-mask (1 valid)
        m["jbias"] = np.ascontiguousarray((NEG * (1.0 - mp)).reshape(1, L)).astype(BF)
        m["mbias"] = np.ascontiguousarray((NEG * (1.0 - mp)).reshape(L, 1))
        mi = np.asarray(mask[b, i0 : i0 + LI], f)
        m["miof"] = np.ascontiguousarray(
            np.stack([mi, (1.0 - mi) * (2.0 / L)], axis=1)
        )
        in_maps.append(m)
    return in_maps, per_batch, LI


def kernel(x, mask, refCov, wq, bq, wk, bk, wv, bv, wo, bo,
           r1w, r1b, r2w, r2b, ln_g, ln_b, trace=False):
    x = np.asarray(x)
    Bc, L, Hc = x.shape
    flags = (
        bool(np.any(bq)), bool(np.any(bk)), bool(np.any(bv)), bool(np.any(bo)),
        bool(np.any(r2b)),
    )
    in_maps, per_batch, LI = make_in_maps(
        x, mask, refCov, wq, bq, wk, bk, wv, bv, wo, bo,
        r1w, r1b, r2w, r2b, ln_g, ln_b,
    )
    nc = _get_program(L, LI, flags, [float(v) for v in np.asarray(r2b).ravel()])
    res = run_bass_kernel_spmd(nc, in_maps, core_ids=list(range(N_CORES)), trace=trace)
    out = np.empty((Bc, L, Hc), np.float32)
    for c in range(N_CORES):
        b, half = c // per_batch, c % per_batch
        out[b, half * LI : (half + 1) * LI] = res.results[c]["y"]
    if trace:
        return out, res
    return out



# revision 81
# speedup vs baseline: 1771.7041x; 1771.7041x over previous
"""Trainium2 Bass kernel for nn_AttentionBlock (sparse_attention).

Full-input contract: kernel(**inputs) takes the complete tensors and returns
the complete [4, 512, 512] output. Internally shards over 8 NeuronCores as
(batch, i-half): core c handles batch c//2, query rows (c%2)*256 ..+256.

Key host-side preprocessing:
- context (j) packing: only mask-valid context columns are kept (padded to
  a multiple of 128 shared across cores), shrinking every j-sized tensor /
  matmul / exp by ~25%; the reference's uniform-attention rows for invalid
  queries are reproduced exactly by a host-computed rank-1 term
  (ubar = PS*(2/L)*sum_j x_j @ wv, outer zrow = 1-mask_i) accumulated into
  the AV psum.
- refCov is packed, pre-permuted AND pre-transposed into the exact
  [2*CIN, (pr2, plane, j)] tile layout the fp8 DoubleRow L1 matmul
  consumes (x32 scale, e4m3); weights r1w ride the same fp8 path with the
  scale folded out through bd2; everything else is bf16.

On-chip pipeline per 16-query-row chunk: fp8 DoubleRow L1 (4 query rows
per matmul at 0.5 cyc/row), relu evacuations split 3:1 DVE:ACT, L2 with
PE-quadrant packing, the j-mask bias accumulated by a rank-1 ones x jbias
matmul straight into the L2 psum, exp straight off psum (no-max softmax,
O(1) logits) with accum_out row sums, fp8 (xPS=64) normalized
probabilities, and ONE deinterleave DMA per chunk (128-partition
interleaved read -> 16-partition head-blocked write, rows padded to 512B
for full DMA descriptor rate). Chunk tails are emitted one chunk deferred
so no engine queue head-of-line-blocks on the exp. Attention scores fold
the j-mask via a K=1 bias-row accumulate; the combine (attn+ref softmax
sum) is a single GPSIMD add; AV transposes evacuate via DVE (ACT in the
final-block tail); the output projection's wo matmuls accumulate
incrementally as head pairs finish, and layernorm runs bn_stats/bn_aggr +
magic-rsqrt on DVE with the affine on GPSIMD in the tail. DMA rings:
refc stream and deinterleaves alternate SP/GPSIMD; small constants ride
ACT; x/weights follow the first refc prefetches.

Self-contained: hardcodes all shapes; no sibling imports.
"""

import sys

if "/opt/trn_rl_repo" not in sys.path:
    sys.path.insert(0, "/opt/trn_rl_repo")

from collections import deque
from contextlib import ExitStack

import ml_dtypes
import numpy as np

import concourse.bass as bass
import concourse.tile as tile
from concourse import bacc, mybir
from concourse.bass_utils import run_bass_kernel_spmd
from concourse.masks import make_identity

F32 = mybir.dt.float32
BF16 = mybir.dt.bfloat16
FP8 = mybir.dt.float8e4
I32 = mybir.dt.int32
AF = mybir.ActivationFunctionType
ALU = mybir.AluOpType
AX = mybir.AxisListType
MPM = mybir.MatmulPerfMode

BF = ml_dtypes.bfloat16
F8 = ml_dtypes.float8_e4m3
NEG = -1.0e30

B, L_FULL, H, NH = 4, 512, 512, 8
DK = H // NH  # 64
CIN, CHID = 53, 32
N_CORES = 8
PRE = 3  # refc chunk DMA prefetch depth
RCS = 32.0   # refc fp8 pre-scale (folded out via bd2)
PS = 64.0    # probability fp8 pre-scale (folded out in avT evac)


def build_program(L, LI, has_bq, has_bk, has_bv, has_bo, has_r2b, r2b_vals,
                  trace_sim=False, n_reps=1):
    """One-core program: attention block over LI query rows, L context.

    n_reps>1 repeats the whole body (refc stream + qkv + heads + proj) for
    wall-clock A/B timing; weights/x loads happen once."""
    assert L % 128 == 0 and LI % 16 == 0
    NJB = L // 128            # j blocks
    IBS = min(128, LI)        # i-block size for attention tiles
    NIB = LI // IBS           # i-blocks
    NHC = H // 128            # h chunks (4)
    NCH = LI // 16            # refc chunks (16 i-rows each)
    CPI = IBS // 16           # chunks per i-block
    RCF = 8 * NJB * 128       # refct free elems per partition row (per chunk)
    BIGF = max(L, H)          # "big" psum tile free size
    LDP = max(L, 512)         # deint row padded to >=512B (fp8) for DMA rate
    scale = float(1.0 / np.sqrt(DK))

    nc = bacc.Bacc()

    xb_d = nc.dram_tensor("xb", [L, H], BF16, kind="ExternalInput")
    xqb_d = nc.dram_tensor("xqb", [LI, H], BF16, kind="ExternalInput")
    xq_d = nc.dram_tensor("xq", [LI, H], F32, kind="ExternalInput")
    ubar_d = nc.dram_tensor("ubar", [1, H], BF16, kind="ExternalInput")
    zrow_d = nc.dram_tensor("zrow", [1, LI], BF16, kind="ExternalInput")
    refct_d = nc.dram_tensor("refct", [NCH * 2 * CIN, RCF], FP8,
                             kind="ExternalInput")
    wqkvo_d = nc.dram_tensor("wqkvo", [H, 4 * H], BF16, kind="ExternalInput")
    bqkvo_d = nc.dram_tensor("bqkvo", [H, 4], F32, kind="ExternalInput")
    bd1_d = nc.dram_tensor("bd1h", [2 * CIN, 2 * 128], FP8, kind="ExternalInput")
    bd2_d = nc.dram_tensor("bd2h", [128, 4 * NH], BF16, kind="ExternalInput")
    r1b4_d = nc.dram_tensor("r1b4h", [128, 1], F32, kind="ExternalInput")
    jbias_d = nc.dram_tensor("jbias", [1, L], BF16, kind="ExternalInput")
    mbias_d = nc.dram_tensor("mbias", [L, 1], F32, kind="ExternalInput")
    miof_d = nc.dram_tensor("miof", [LI, 2], F32, kind="ExternalInput")
    miint_d = nc.dram_tensor("miint", [128, NCH], F32, kind="ExternalInput")
    lng_d = nc.dram_tensor("lng", [H, 1], F32, kind="ExternalInput")
    lnb_d = nc.dram_tensor("lnb", [H, 1], F32, kind="ExternalInput")
    y_d = nc.dram_tensor("y", [LI, H], F32, kind="ExternalOutput")

    with tile.TileContext(nc, trace_sim=trace_sim) as tc, ExitStack() as ctx:
        P = ctx.enter_context(tc.tile_pool(name="persist", bufs=1))
        # per-rep "persistent" tiles; ring of 2 when the body repeats
        PR = ctx.enter_context(
            tc.tile_pool(name="repP", bufs=(1 if n_reps == 1 else 2)))
        pref = ctx.enter_context(tc.tile_pool(name="rct", bufs=PRE + 2))
        phid = ctx.enter_context(tc.tile_pool(name="hid", bufs=3))
        pl2s = ctx.enter_context(tc.tile_pool(name="l2s", bufs=3))
        perl = ctx.enter_context(tc.tile_pool(name="erl", bufs=3))
        pea = ctx.enter_context(tc.tile_pool(name="ea", bufs=4))
        per = ctx.enter_context(tc.tile_pool(name="er", bufs=5))
        pwts = ctx.enter_context(tc.tile_pool(name="wts", bufs=4))
        pry = ctx.enter_context(tc.tile_pool(name="ry", bufs=2))
        psc = ctx.enter_context(tc.tile_pool(name="sc", bufs=6))
        # PSUM pools (2KB banks): big 2 + l1 3 + l2 2 + wav 1 = 8
        pp_big = ctx.enter_context(tc.tile_pool(name="ppbig", bufs=2, space="PSUM"))
        pp_l1 = ctx.enter_context(tc.tile_pool(name="ppl1", bufs=3, space="PSUM"))
        pp_l2 = ctx.enter_context(tc.tile_pool(name="ppl2", bufs=2, space="PSUM"))
        pp_wav = ctx.enter_context(tc.tile_pool(name="ppwav", bufs=1, space="PSUM"))
        pp_tr = pp_big  # x transposes (startup) ride the big pool

        # ---------- phase 0: identity + x loads (ACT queue; SP is for refct) -
        ident = P.tile([128, 128], F32, tag="ident")
        make_identity(nc, ident)
        identb = P.tile([128, 128], BF16, tag="identb")
        nc.vector.tensor_copy(out=identb, in_=ident)

        xf = [P.tile([128, H], BF16, tag=f"xf{t}", name=f"xf{t}")
              for t in range(NJB)]
        xqb_sb = [P.tile([IBS, H], BF16, tag=f"xqb{ib}", name=f"xqb{ib}")
                  for ib in range(NIB)]
        xq_sb = [P.tile([IBS, H], F32, tag=f"xq{ib}", name=f"xq{ib}")
                 for ib in range(NIB)]

        def emit_x_loads():
            for t in range(NJB):
                # row p of tile t = x row j = NJB*p + t (pi-permuted j order)
                eng = nc.sync if t % 2 == 0 else nc.gpsimd
                eng.dma_start(out=xf[t], in_=xb_d[t : L : NJB, :])
            for ib in range(NIB):
                nc.gpsimd.dma_start(
                    out=xqb_sb[ib], in_=xqb_d[IBS * ib : IBS * (ib + 1), :])
                nc.sync.dma_start(
                    out=xq_sb[ib], in_=xq_d[IBS * ib : IBS * (ib + 1), :])

        # ---------- weights + small constants (all pre-cast on host) --------
        def w_sb(nm, kk):
            wi = ("wq", "wk", "wv", "wo").index(nm)
            return wbig[kk][:, H * wi : H * (wi + 1)]

        # critical-path constants first on the ACT queue
        jbias_b = P.tile([1, L], BF16, tag="jbias_b")
        nc.scalar.dma_start(out=jbias_b, in_=jbias_d[:, :])
        bd1b = P.tile([2 * CIN, 2, 128], FP8, tag="bd1b")
        nc.scalar.dma_start(
            out=bd1b.rearrange("p q m -> p (q m)"), in_=bd1_d[:, :])
        bd2b = P.tile([128, 4 * NH], BF16, tag="bd2b")
        nc.scalar.dma_start(out=bd2b, in_=bd2_d[:, :])
        r1b4 = P.tile([128, 1], F32, tag="r1b4")
        nc.scalar.dma_start(out=r1b4, in_=r1b4_d[:, :])
        miint = P.tile([128, NCH], F32, tag="miint")
        nc.scalar.dma_start(out=miint, in_=miint_d[:, :])
        ones1b = P.tile([1, 128], BF16, tag="ones1b")
        nc.gpsimd.memset(ones1b, 1.0)

        # later-needed constants ride the SP queue, emitted after the first
        # refc prefetches (see emit_late_consts)
        ubar_t = P.tile([1, H], BF16, tag="ubar_t")
        zrow_t = P.tile([1, LI], BF16, tag="zrow_t")
        miof = [P.tile([IBS, 2], F32, tag=f"miof{ib}", name=f"miof{ib}")
                for ib in range(NIB)]
        g_bc = P.tile([128, H], F32, tag="g_bc")
        b_bc = P.tile([128, H], F32, tag="b_bc")
        bo_bc = None
        r2b128 = None
        if has_bo:
            bo_d2 = nc.dram_tensor("bo2", [H, 1], F32, kind="ExternalInput")
            bo_bc = P.tile([128, H], F32, tag="bo_bc")
        if has_r2b:
            r2b128 = P.tile([128, 1], F32, tag="r2b128")
            r2bh_d = nc.dram_tensor("r2b128h", [128, 1], F32, kind="ExternalInput")

        def emit_late_consts():
            nc.sync.dma_start(out=ubar_t, in_=ubar_d[:, :])
            nc.sync.dma_start(out=zrow_t, in_=zrow_d[:, :])
            for ib in range(NIB):
                nc.sync.dma_start(
                    out=miof[ib], in_=miof_d[IBS * ib : IBS * (ib + 1), :])
            nc.sync.dma_start(
                out=g_bc, in_=bass.AP(tensor=lng_d, offset=0, ap=[[0, 128], [1, H]])
            )
            nc.sync.dma_start(
                out=b_bc, in_=bass.AP(tensor=lnb_d, offset=0, ap=[[0, 128], [1, H]])
            )
            if has_bo:
                nc.sync.dma_start(
                    out=bo_bc,
                    in_=bass.AP(tensor=bo_d2, offset=0, ap=[[0, 128], [1, H]]),
                )
            if has_r2b:
                nc.sync.dma_start(out=r2b128, in_=r2bh_d[:, :])

        bias_sb = {}
        if has_bq or has_bk or has_bv:
            bqkvo = P.tile([128, NHC, 4], F32, tag="bqkvo")
            nc.scalar.dma_start(
                out=bqkvo, in_=bqkvo_d[:, :].rearrange("(c p) b -> p c b", p=128)
            )
            for wi, nm in enumerate(("bq", "bk", "bv")):
                for kk in range(NHC):
                    bias_sb[(nm, kk)] = bqkvo[:, kk, wi : wi + 1]

        wbig = [P.tile([128, 4 * H], BF16, tag=f"wbig{kk}", name=f"wbig{kk}")
                for kk in range(NHC)]

        def emit_w_loads():
            for kk in range(NHC):
                eng = nc.sync if kk % 2 == 0 else nc.gpsimd
                eng.dma_start(
                    out=wbig[kk], in_=wqkvo_d[128 * kk : 128 * (kk + 1), :])

        # ---------- x transposes (emitted inside rep 0, after chunk 0) ------
        xT = []
        xqT = []

        def emit_xtr():
            for hc in range(NHC):
                ps = pp_tr.tile([128, L], BF16, tag="big", name="psT")
                for jt in range(NJB):
                    nc.tensor.transpose(
                        out=ps[:, 128 * jt : 128 * (jt + 1)],
                        in_=xf[jt][:, 128 * hc : 128 * (hc + 1)],
                        identity=identb,
                    )
                xs = P.tile([128, L], BF16, tag=f"xT{hc}")
                nc.vector.tensor_copy(out=xs, in_=ps)
                xT.append(xs)
            for hc in range(NHC):
                ps = pp_tr.tile([128, L], BF16, tag="big", name="psT")
                for ib in range(NIB):
                    nc.tensor.transpose(
                        out=ps[:, IBS * ib : IBS * (ib + 1)],
                        in_=xqb_sb[ib][:, 128 * hc : 128 * (hc + 1)],
                        identity=identb[0:IBS, 0:IBS],
                    )
                xs = P.tile([128, LI], BF16, tag=f"xqT{hc}")
                nc.vector.tensor_copy(out=xs, in_=ps[:, 0:LI])
                xqT.append(xs)

        def rsqrt_dve(out, v):
            """out = 1/sqrt(v) on DVE only (magic seed + Newton steps)."""
            yb = psc.tile([IBS, 1], I32, tag="rsq_i")
            nc.vector.tensor_scalar(
                out=yb, in0=v.bitcast(I32), scalar1=1, scalar2=None,
                op0=ALU.logical_shift_right,
            )
            nc.vector.tensor_scalar(
                out=yb, in0=yb, scalar1=-1, scalar2=0x5F3759DF,
                op0=ALU.mult, op1=ALU.add,
            )
            y = yb.bitcast(F32)
            t2 = psc.tile([IBS, 1], F32, tag="rsq_t")
            for _ in range(2):
                nc.vector.tensor_mul(out=t2, in0=y, in1=y)
                nc.vector.tensor_mul(out=t2, in0=t2, in1=v)
                nc.vector.tensor_scalar(
                    out=t2, in0=t2, scalar1=-0.5, scalar2=1.5,
                    op0=ALU.mult, op1=ALU.add,
                )
                nc.vector.tensor_mul(out=y, in0=y, in1=t2)
            nc.vector.tensor_copy(out=out, in_=y)

        def rsqrt_dve_fast(out, v):
            """1/sqrt(v) on DVE: magic seed + one Newton step (~0.2% rel
            err; the layernorm tolerance absorbs it)."""
            yb = psc.tile([IBS, 1], I32, tag="rsq_i")
            nc.vector.tensor_scalar(
                out=yb, in0=v.bitcast(I32), scalar1=1, scalar2=None,
                op0=ALU.logical_shift_right,
            )
            nc.vector.tensor_scalar(
                out=yb, in0=yb, scalar1=-1, scalar2=0x5F3759DF,
                op0=ALU.mult, op1=ALU.add,
            )
            y = yb.bitcast(F32)
            t2 = psc.tile([IBS, 1], F32, tag="rsq_t")
            nc.vector.tensor_mul(out=t2, in0=y, in1=y)
            nc.vector.tensor_mul(out=t2, in0=t2, in1=v)
            nc.vector.tensor_scalar(
                out=t2, in0=t2, scalar1=-0.5, scalar2=1.5,
                op0=ALU.mult, op1=ALU.add,
            )
            nc.vector.tensor_mul(out=out, in0=y, in1=t2)

        # ================= per-rep body =================
        def emit_rep(rep):
            # ---------- refc chunk prefetch machinery ----------
            rct_q = deque()

            def issue_rct(c):
                t = pref.tile([2 * CIN, 4, 2, NJB * 128], FP8, tag="rct", name="rct")
                eng = nc.sync if c % 2 == 0 else nc.gpsimd
                eng.dma_start(
                    out=t.rearrange("p a q j -> p (a q j)"),
                    in_=refct_d[106 * c : 106 * (c + 1), :],
                )
                rct_q.append(t)

            for c in range(min(PRE, NCH)):
                issue_rct(c)
            if rep == 0:
                emit_x_loads()
                emit_w_loads()
                emit_late_consts()

            # ---------- q/k/v projections (bf16), emitted after chunk 0 -----
            qT, kT, v_sb = [], [], []

            def emit_qkv():
                for t in range(NHC):
                    ps = pp_big.tile([128, BIGF], F32, tag="big", name="psq")
                    for kk in range(NHC):
                        nc.tensor.matmul(
                            out=ps[:, 0:LI],
                            lhsT=w_sb("wq", kk)[:, 128 * t : 128 * (t + 1)],
                            rhs=xqT[kk], start=(kk == 0), stop=(kk == NHC - 1),
                        )
                    s = PR.tile([128, LI], BF16, tag=f"qT{t}")
                    if has_bq:
                        nc.scalar.activation(out=s, in_=ps[:, 0:LI], func=AF.Identity,
                                             bias=bias_sb[("bq", t)])
                    else:
                        nc.scalar.copy(out=s, in_=ps[:, 0:LI])
                    qT.append(s)
                for t in range(NHC):
                    ps = pp_big.tile([128, BIGF], F32, tag="big", name="psk")
                    for kk in range(NHC):
                        nc.tensor.matmul(
                            out=ps[:, 0:L],
                            lhsT=w_sb("wk", kk)[:, 128 * t : 128 * (t + 1)],
                            rhs=xT[kk], start=(kk == 0), stop=(kk == NHC - 1),
                        )
                    s = PR.tile([128, L], BF16, tag=f"kT{t}")
                    if has_bk:
                        nc.scalar.activation(out=s, in_=ps[:, 0:L], func=AF.Identity,
                                             bias=bias_sb[("bk", t)])
                    else:
                        nc.vector.tensor_copy(out=s, in_=ps[:, 0:L])
                    kT.append(s)
                for t in range(NJB):
                    ps = pp_big.tile([128, BIGF], F32, tag="big", name="psv")
                    for kk in range(NHC):
                        nc.tensor.matmul(
                            out=ps[:, 0:H],
                            lhsT=xT[kk][:, 128 * t : 128 * (t + 1)],
                            rhs=w_sb("wv", kk),
                            start=(kk == 0), stop=(kk == NHC - 1),
                        )
                    s = PR.tile([128, H], BF16, tag=f"v{t}")
                    # bv folded into avT evac
                    nc.vector.tensor_copy(out=s, in_=ps[:, 0:H])
                    v_sb.append(s)

            # fp8 normalized ref probabilities (x PS), [i, (head, j)] per i-block
            erS = [
                PR.tile([IBS, NH * LDP], FP8, tag=f"erS{ib}", name=f"erS{ib}_{rep}")
                for ib in range(NIB)
            ]
            aoT = [PR.tile([128, LI], BF16, tag=f"aoT{t}", name=f"aoT{t}_{rep}")
                   for t in range(NHC)]

            def emit_deint(ib, u, erlb):
                # one DMA per chunk: 128-partition (i,h)-interleaved read ->
                # 16-partition write, head-blocked free dim (order matches);
                # rows padded to LDP so descriptors stay >= 512B
                eng = nc.gpsimd if u % 2 == 0 else nc.sync
                eng.dma_start(
                    out=erS[ib][16 * u : 16 * u + 16, :],
                    in_=erlb[:, :],
                )

            def emit_chunk_fore(c):
                """16 i-rows: prefetch DMA + L1 + relu + L2 + mask rank-1."""
                if c + PRE < NCH:
                    issue_rct(c + PRE)
                tr = rct_q.popleft()
                l2p = pp_l2.tile([128, L], F32, tag="l2", name="l2p")
                for pr2 in range(4):
                    l1p = pp_l1.tile([128, BIGF], F32, tag="l1", name="l1p")[:, 0:L]
                    # fp8 DoubleRow: 4 i-rows per matmul (2 k-planes of 106)
                    nc.tensor.matmul(
                        out=l1p, lhsT=bd1b[:, :, :], rhs=tr[:, pr2, :, :],
                        start=True, stop=True, perf_mode=MPM.DoubleRow,
                    )
                    hid = phid.tile([128, L], BF16, tag="hid")
                    # relu evac: 3 of 4 on DVE, 1 on ACT (ACT carries the exps)
                    if pr2 != 1:
                        nc.vector.tensor_scalar(
                            out=hid, in0=l1p, scalar1=r1b4, scalar2=0.0,
                            op0=ALU.add, op1=ALU.max,
                        )
                    else:
                        nc.scalar.activation(
                            out=hid, in_=l1p, func=AF.Relu, bias=r1b4)
                    nc.tensor.matmul(
                        out=l2p[32 * pr2 : 32 * pr2 + 32, :],
                        lhsT=bd2b, rhs=hid, start=True, stop=False,
                        tile_position=(0, 32 * pr2), skip_group_check=True,
                    )
                # accumulate the j-mask bias row over the whole bank (rank-1)
                nc.tensor.matmul(
                    out=l2p, lhsT=ones1b[:, 0:128], rhs=jbias_b,
                    start=False, stop=True, skip_group_check=True,
                )
                return l2p

            def emit_chunk_tail(c, l2p):
                """exp straight off psum, row-normalize, cast fp8, deint."""
                ib = (16 * c) // IBS
                src = l2p
                if has_r2b:
                    l2s = pl2s.tile([128, L], F32, tag="l2s")
                    nc.vector.tensor_scalar_add(out=l2s, in0=l2p, scalar1=r2b128)
                    src = l2s
                erl = perl.tile([128, L], BF16, tag="erl")
                srl = psc.tile([128, 1], F32, tag="srl")
                nc.scalar.activation(
                    out=erl, in_=src, func=AF.Exp, bias=0.0, scale=1.0,
                    accum_out=srl,
                )
                isrl = psc.tile([128, 1], F32, tag="isrl")
                nc.vector.reciprocal(out=isrl, in_=srl)
                nc.vector.tensor_mul(out=isrl, in0=isrl, in1=miint[:, c : c + 1])
                erlb = perl.tile([128, LDP], FP8, tag="erlb")
                nc.gpsimd.tensor_scalar_mul(
                    out=erlb[:, 0:L], in0=erl, scalar1=isrl)
                emit_deint(ib, c % CPI, erlb)

            pend_tail = deque()

            def emit_chunk(c):
                # one-chunk tail deferral: tail(c-1) emits after fore(c), so
                # no engine queue blocks on the exp's upstream chain
                l2p = emit_chunk_fore(c)
                while len(pend_tail) > 1:
                    emit_chunk_tail(*pend_tail.popleft())
                pend_tail.append((c, l2p))

            def drain_chunks():
                while pend_tail:
                    emit_chunk_tail(*pend_tail.popleft())

            eaP = [PR.tile([IBS, L], BF16, tag=f"eaP{nh}", name=f"eaP{nh}_{rep}")
                   for nh in range(NH)]

            def emit_head_attn(ib, nh, store=False):
                """Scores + exp + 1/sa scaling; erS-independent."""
                t, s = nh // 2, nh % 2
                sp = pp_big.tile([IBS, BIGF], F32, tag="big", name="sp")[:, 0:L]
                nc.tensor.matmul(
                    out=sp,
                    lhsT=qT[t][64 * s : 64 * s + 64, IBS * ib : IBS * (ib + 1)],
                    rhs=kT[t][64 * s : 64 * s + 64, :],
                    start=True, stop=False,
                )
                nc.tensor.matmul(
                    out=sp, lhsT=ones1b[:, 0:IBS], rhs=jbias_b,
                    start=False, stop=True, skip_group_check=True,
                )
                ea_t = eaP[nh] if store else pea.tile([IBS, L], BF16, tag="ea")
                sa = psc.tile([IBS, 1], F32, tag="sa")
                nc.scalar.activation(
                    out=ea_t, in_=sp, func=AF.Exp, bias=0.0, scale=scale, accum_out=sa
                )
                isa = psc.tile([IBS, 1], F32, tag="isa")
                nc.vector.reciprocal(out=isa, in_=sa)
                # invalid-i rows zero here; their uniform term is the ubar
                # rank-1 accumulate in the AV matmul
                nc.vector.tensor_mul(out=isa, in0=isa, in1=miof[ib][:, 0:1])
                nc.gpsimd.tensor_scalar_mul(out=ea_t, in0=ea_t, scalar1=isa)
                return ea_t

            def emit_head_ref_a(ib, nh, ea_t):
                """combine: w = erS_head + ea (single GPSIMD add)."""
                er_t = per.tile([IBS, L], BF16, tag="er")
                nc.gpsimd.tensor_add(
                    out=er_t, in0=erS[ib][:, LDP * nh : LDP * nh + L], in1=ea_t
                )
                return er_t

            def emit_head_ref_b(ib, nh, ea_t, alt=False):
                """wT transpose + AV (PE heavy). In the tail (alt) the psum
                evacs ride ACT, which is idle there."""
                t, s = nh // 2, nh % 2
                odd = alt and (nh % 2 == 1)
                wtp = (pp_l2 if odd else pp_wav).tile(
                    [128, NJB * IBS], BF16, tag=("l2" if odd else "wav"), name="wtp")
                for k in range(NJB):
                    nc.tensor.transpose(
                        out=wtp[:, IBS * k : IBS * (k + 1)],
                        in_=ea_t[:, 128 * k : 128 * (k + 1)],
                        identity=identb[0:IBS, 0:IBS],
                    )
                wts = pwts.tile([128, NJB * IBS], BF16, tag="wts")
                if alt and nh % 2 == 0:
                    nc.scalar.copy(out=wts, in_=wtp)
                else:
                    nc.vector.tensor_copy(out=wts, in_=wtp)
                avp = (pp_l2 if odd else pp_wav).tile(
                    [64, IBS], F32, tag=("l2" if odd else "wav"), name="avp")
                for k in range(NJB):
                    nc.tensor.matmul(
                        out=avp,
                        lhsT=v_sb[k][:, 64 * nh : 64 * nh + 64],
                        rhs=wts[:, IBS * k : IBS * (k + 1)],
                        start=(k == 0), stop=False,
                    )
                # uniform row for invalid i: += ubar^T zrow (rank-1)
                nc.tensor.matmul(
                    out=avp, lhsT=ubar_t[:, 64 * nh : 64 * nh + 64],
                    rhs=zrow_t[:, IBS * ib : IBS * (ib + 1)],
                    start=False, stop=True, skip_group_check=True,
                )
                ao_slice = aoT[t][64 * s : 64 * s + 64, IBS * ib : IBS * (ib + 1)]
                if has_bv:
                    nc.scalar.activation(
                        out=ao_slice, in_=avp, func=AF.Identity, scale=0.5 / PS,
                        bias=bias_sb[("bv", t)][64 * s : 64 * s + 64, :],
                    )
                elif alt and nh % 2 == 1:
                    nc.scalar.activation(
                        out=ao_slice, in_=avp, func=AF.Copy, bias=0.0,
                        scale=0.5 / PS,
                    )
                else:
                    nc.vector.tensor_scalar_mul(
                        out=ao_slice, in0=avp, scalar1=0.5 / PS,
                    )

            def emit_proj_mm(pp, ib, kk, n0=0):
                nc.tensor.matmul(
                    out=pp[:, 0:H],
                    lhsT=aoT[kk][:, IBS * ib : IBS * (ib + 1)],
                    rhs=w_sb("wo", kk),
                    start=(kk == n0), stop=(kk == NHC - 1),
                )

            def emit_proj_tail(pp, ib, tail=False):
                r_t = pry.tile([IBS, H], F32, tag="rt")
                nc.vector.scalar_tensor_tensor(
                    out=r_t, in0=pp[:, 0:H], scalar=1.0, in1=xq_sb[ib],
                    op0=ALU.mult, op1=ALU.add,
                )
                if has_bo:
                    nc.vector.tensor_add(out=r_t, in0=r_t, in1=bo_bc[0:IBS, :])
                stats = psc.tile([IBS, 6], F32, tag="stats")
                nc.vector.bn_stats(out=stats, in_=r_t)
                mv = psc.tile([IBS, 2], F32, tag="mv")
                nc.vector.bn_aggr(out=mv, in_=stats)
                # in the tail, the scale/affine chain rides Pool (DVE is busy)
                ve = nc.gpsimd if tail else nc.vector
                veps = psc.tile([IBS, 1], F32, tag="veps")
                nc.vector.tensor_scalar_add(out=veps, in0=mv[:, 1:2], scalar1=1e-5)
                rstd = psc.tile([IBS, 1], F32, tag="rstd")
                rsqrt_dve_fast(rstd, veps)
                ve.tensor_scalar(
                    out=r_t, in0=r_t, scalar1=mv[:, 0:1], scalar2=rstd,
                    op0=ALU.subtract, op1=ALU.mult,
                )
                y_t = pry.tile([IBS, H], F32, tag="yt")
                ve.tensor_mul(out=y_t, in0=r_t, in1=g_bc[0:IBS, :])
                ve.tensor_add(out=y_t, in0=y_t, in1=b_bc[0:IBS, :])
                nc.sync.dma_start(out=y_d[IBS * ib : IBS * (ib + 1), :], in_=y_t)

            def emit_proj(ib):
                pp = pp_big.tile([IBS, BIGF], F32, tag="big", name="pp")
                for kk in range(NHC):
                    emit_proj_mm(pp, ib, kk)
                emit_proj_tail(pp, ib)

            # ---------- schedule ----------
            emit_chunk(0)
            if rep == 0:
                emit_xtr()
            emit_qkv()
            for c in range(1, CPI):
                emit_chunk(c)
            last = NIB - 1
            for ib in range(NIB):
                drain_chunks()  # erS[ib] deints must be emitted before reads
                nxt = list(range(CPI * (ib + 1), min(CPI * (ib + 2), NCH)))
                tail = (ib == last and NIB > 1)
                # in the tail, pp_big is free (no sp allocations), so the wo
                # matmuls accumulate incrementally as head pairs finish
                pp_hold = (pp_big.tile([IBS, BIGF], F32, tag="big", name="ppt")
                           if tail else None)
                for nh in range(NH):
                    if tail:
                        ea_t = eaP[nh]
                    else:
                        ea_t = emit_head_attn(ib, nh)
                    w_t = emit_head_ref_a(ib, nh, ea_t)
                    # front-load the next block's chunks (4 per head) so its
                    # erS completes well before its head loop starts
                    for _ in range(4):
                        if nxt:
                            emit_chunk(nxt.pop(0))
                    emit_head_ref_b(ib, nh, w_t, alt=tail)
                    if tail and nh % 2 == 1:
                        emit_proj_mm(pp_hold, ib, nh // 2, n0=0)
                    if ib + 1 == last and NIB > 1:
                        # precompute the final i-block's attn side early
                        emit_head_attn(last, nh, store=True)
                while nxt:
                    emit_chunk(nxt.pop(0))
                if not nxt:
                    drain_chunks()
                if tail:
                    emit_proj_tail(pp_hold, ib, tail=True)
                else:
                    emit_proj(ib)

        for rep in range(n_reps):
            emit_rep(rep)

    nc.compile()
    return nc


def _make_bd1q(r1w):
    """fp8 DoubleRow L1 weights [2*CIN, 2, 128]: plane q, out col 32*g+h
    carries r1w[cc, h] for i-row g = 2*i2 + q (block-diag over i2)."""
    bd1 = np.zeros((2 * CIN, 2, 128), np.float32)
    for i2 in range(2):
        for q in range(2):
            g = 2 * i2 + q
            bd1[CIN * i2 : CIN * (i2 + 1), q, 32 * g : 32 * g + CHID] = r1w
    return np.clip(bd1, -240, 240).astype(F8).reshape(2 * CIN, 2 * 128)


def _make_bd2(r2w):
    bd2 = np.zeros((128, 4 * NH), np.float32)
    for g in range(4):
        bd2[32 * g : 32 * g + CHID, NH * g : NH * (g + 1)] = r2w / RCS
    return bd2


def _make_r1b4(r1b):
    r1b4 = np.zeros((128, 1), np.float32)
    for g in range(4):
        r1b4[32 * g : 32 * g + CHID, 0] = r1b * RCS
    return r1b4


def _pack_refct(rc, L):
    """[LI, L, CIN] f32 -> [(LI/16)*106, 8*NJB*128] fp8 (x RCS), pre-transposed
    so the fp8 DoubleRow L1 matmul consumes it directly: chunk c rows
    (i2, cc), cols (pr2, q, jb, p) with j = p*NJB + jb (pi-permuted j order)
    and i-row = 16c + 4*pr2 + 2*i2 + q."""
    LI = rc.shape[0]
    NJB = L // 128
    nch = LI // 16
    a = rc.reshape(nch, 4, 2, 2, 128, NJB, CIN)  # (c, pr2, i2, q, p, jb, cc)
    a = a.transpose(0, 2, 6, 1, 3, 5, 4)         # (c, i2, cc, pr2, q, jb, p)
    a = np.clip(a * RCS, -240, 240)
    return np.ascontiguousarray(
        a.reshape(nch * 2 * CIN, 8 * NJB * 128).astype(F8)
    )


def _make_miint(mi, NCH):
    """[128, NCH] f32: PS * valid-i mask, (pr2, g, nh)-interleaved layout."""
    p = np.arange(128)
    out = np.zeros((128, NCH), np.float32)
    for c in range(NCH):
        out[:, c] = PS * mi[16 * c + 4 * (p // 32) + (p % 32) // 8]
    return out


_PROG_CACHE = {}


def _get_program(L, LI, flags, r2b_vals, n_reps=1):
    key = (L, LI, flags, n_reps)
    if key not in _PROG_CACHE:
        _PROG_CACHE[key] = build_program(L, LI, *flags, r2b_vals, n_reps=n_reps)
    return _PROG_CACHE[key]


def make_in_maps(x, mask, refCov, wq, bq, wk, bk, wv, bv, wo, bo,
                 r1w, r1b, r2w, r2b, ln_g, ln_b, n_cores=N_CORES, LI=None):
    """Pack the context (j) axis to the valid-mask columns, padded to LP
    (multiple of 128, shared across cores for SPMD)."""
    Bc, L, Hc = x.shape
    if LI is None:
        LI = (Bc * L) // n_cores
    f = np.float32
    maskb = np.asarray(mask).astype(bool)
    valid = [np.where(maskb[b])[0] for b in range(Bc)]
    LV = max(1, max(len(v) for v in valid))
    LP = ((LV + 127) // 128) * 128
    shared = {
        "wqkvo": np.ascontiguousarray(
            np.concatenate([np.asarray(w, f) for w in (wq, wk, wv, wo)], axis=1)
        ).astype(BF),
        "bqkvo": np.ascontiguousarray(
            np.stack([np.asarray(b, f) for b in (bq, bk, bv, bo)], axis=1)
        ),
        "bd1h": _make_bd1q(np.asarray(r1w, f)),
        "bd2h": _make_bd2(np.asarray(r2w, f)).astype(BF),
        "r1b4h": _make_r1b4(np.asarray(r1b, f)),
        "lng": np.ascontiguousarray(ln_g, f).reshape(Hc, 1),
        "lnb": np.ascontiguousarray(ln_b, f).reshape(Hc, 1),
    }
    njb = LP // 128
    pidx = (np.arange(LP) % 128) * njb + np.arange(LP) // 128
    per_batch = L // LI
    NCH = LI // 16
    wv_f = np.asarray(wv, f)
    bv_f = np.asarray(bv, f)
    in_maps = []
    for c in range(n_cores):
        b, half = c // per_batch, c % per_batch
        i0 = half * LI
        m = dict(shared)
        xb = np.asarray(x[b], f)
        vj = valid[b]
        nv = len(vj)
        # packed context rows (padding rows zero)
        xk = np.zeros((LP, Hc), f)
        xk[:nv] = xb[vj]
        m["xb"] = np.ascontiguousarray(xk).astype(BF)
        m["xqb"] = np.ascontiguousarray(xb[i0 : i0 + LI]).astype(BF)
        m["xq"] = np.ascontiguousarray(xb[i0 : i0 + LI])
        rc = np.zeros((LI, LP, CIN), f)
        rc[:, :nv] = np.asarray(refCov[b, i0 : i0 + LI], f)[:, vj]
        m["refct"] = _pack_refct(rc, LP)
        mp = (np.arange(LP) < nv).astype(f)[pidx]  # permuted packed validity
        m["jbias"] = np.ascontiguousarray((NEG * (1.0 - mp)).reshape(1, LP)).astype(BF)
        m["mbias"] = np.ascontiguousarray((NEG * (1.0 - mp)).reshape(LP, 1))
        mi = np.asarray(mask[b, i0 : i0 + LI], f)
        m["miof"] = np.ascontiguousarray(
            np.stack([PS * mi, 0.0 * mi], axis=1)
        )
        m["miint"] = _make_miint(mi, NCH)
        # uniform-attention row for invalid i: (1/L)*sum_all_j v_j, via a
        # host-computed rank-1 term PS*(2/L)*(sum_j x_j @ wv + L*bv)
        ub = PS * (2.0 / L) * (xb.sum(0) @ wv_f)  # bv added by the avT evac
        m["ubar"] = np.ascontiguousarray(ub.reshape(1, Hc)).astype(BF)
        m["zrow"] = np.ascontiguousarray((1.0 - mi).reshape(1, LI)).astype(BF)
        in_maps.append(m)
    return in_maps, per_batch, LI, LP


def kernel(x, mask, refCov, wq, bq, wk, bk, wv, bv, wo, bo,
           r1w, r1b, r2w, r2b, ln_g, ln_b, trace=False):
    x = np.asarray(x)
    Bc, L, Hc = x.shape
    flags = (
        bool(np.any(bq)), bool(np.any(bk)), bool(np.any(bv)), bool(np.any(bo)),
        bool(np.any(r2b)),
    )
    in_maps, per_batch, LI, LP = make_in_maps(
        x, mask, refCov, wq, bq, wk, bk, wv, bv, wo, bo,
        r1w, r1b, r2w, r2b, ln_g, ln_b,
    )
    nc = _get_program(LP, LI, flags, [float(v) for v in np.asarray(r2b).ravel()])
    res = run_bass_kernel_spmd(nc, in_maps, core_ids=list(range(N_CORES)), trace=trace)
    out = np.empty((Bc, L, Hc), np.float32)
    for c in range(N_CORES):
        b, half = c // per_batch, c % per_batch
        out[b, half * LI : (half + 1) * LI] = res.results[c]["y"]
    if trace:
        return out, res
    return out
